# revision 1
# baseline (speedup 1.0000x reference)
"""Trainium2 Bass kernel for nn_MHA_36584531427723.

Sharding: 8 cores = 2 batches x 4 head-groups (4 heads of 64 dims each per
core). Each core computes its batch's Q/K/V projections restricted to its
head-group's 256 output features, attention for its 4 heads, and a partial
output projection (its 256 rows of Wo^T). The host sums the 4 partials per
batch and adds bo.

Device layout choices (all host-prepped, no on-device transposes):
  - QT/KT = Q[b].T, K[b].T   [1024, 2048] f32  (feature on partitions)
  - projections produce Q_^T/K_^T [256, 2048] (bf16) and V [2048, 4, 65] bf16
    with a ones column at index 64 so the PV matmul also yields the softmax
    denominator row.
  - scores are computed transposed, E^T [k, q], so exp/mask/PV all use
    natural slices; mask is shipped pre-transposed as bf16 0/1.
  - softmax: max-subtraction dropped (|E|<~1 so exp is safe; the reference's
    max shift cancels exactly up to its eps term, relative effect ~1e-11);
    eps dropped (eps/S ~ 1e-11).
"""

import numpy as np
import ml_dtypes

import concourse.bacc as bacc
import concourse.bass as bass  # noqa: F401
import concourse.mybir as mybir
import concourse.tile as tile
from concourse.bass_utils import run_bass_kernel_spmd

B, N, D = 2, 2048, 1024
H = 16
HD = 64
HL = 4  # heads per core
DL = HL * HD  # 256 local features
P = 128
KO = D // P  # 8 contraction chunks for projections
NKC = N // P  # 16 k-token chunks
NQC = N // P
NPAN = 4
PANW = N // NPAN  # 512-wide token panels in the projection phase
SCALE = 1.0 / 32.0  # 1/sqrt(DIM_V)

F32 = mybir.dt.float32
F32R = mybir.dt.float32r
BF16 = mybir.dt.bfloat16
AF = mybir.ActivationFunctionType

def build_nc():
    nc = bacc.Bacc(None, target_bir_lowering=False)
    QT = nc.dram_tensor("qt", (D, N), BF16, kind="ExternalInput")
    KT = nc.dram_tensor("kt", (D, N), BF16, kind="ExternalInput")
    MT = nc.dram_tensor("mt", (N, N), BF16, kind="ExternalInput")
    WQT = nc.dram_tensor("wqt", (D, DL), BF16, kind="ExternalInput")
    WKT = nc.dram_tensor("wkt", (D, DL), BF16, kind="ExternalInput")
    WVT = nc.dram_tensor("wvt", (D, DL), BF16, kind="ExternalInput")
    WOT = nc.dram_tensor("wot", (DL, D), BF16, kind="ExternalInput")
    BQ = nc.dram_tensor("bq", (DL,), F32, kind="ExternalInput")
    BK = nc.dram_tensor("bk", (DL,), F32, kind="ExternalInput")
    BV = nc.dram_tensor("bv", (DL,), F32, kind="ExternalInput")
    OUT = nc.dram_tensor("out", (N, D), F32, kind="ExternalOutput")

    qt_r = QT[:].rearrange("(ko p) q -> p ko q", p=P)
    kt_r = KT[:].rearrange("(ko p) q -> p ko q", p=P)
    mt_r = MT[:].rearrange("(kc p) q -> p kc q", p=P)

    with tile.TileContext(nc) as tc:
        with (
            tc.tile_pool(name="persist", bufs=1) as persist,
            tc.tile_pool(name="otpool", bufs=1) as otpool,
        ):
            # --- persistent tiles ---
            mT = persist.tile([P, NKC, N], BF16)  # 64KB/part
            qT = persist.tile([P, 2, N], BF16, tag="qT")  # Q_^T, 8KB
            kT = persist.tile([P, 2, N], BF16, tag="kT")
            v_sb = persist.tile([P, NKC, HL, HD + 1], BF16, tag="v")
            ones_sb = persist.tile([1, HD], F32, tag="ones")
            nc.vector.memset(ones_sb[:], 1.0)
            bq_sb = persist.tile([P, 2], F32, tag="bq")
            bk_sb = persist.tile([P, 2], F32, tag="bk")
            bv_rep = persist.tile([P, HL, HD], F32, tag="bv")
            wo_sb = persist.tile([P, 2, D], BF16, tag="wo")

            nc.sync.dma_start(out=bq_sb[:], in_=BQ[:].rearrange("(c p) -> p c", p=P))
            nc.sync.dma_start(out=bk_sb[:], in_=BK[:].rearrange("(c p) -> p c", p=P))
            nc.sync.dma_start(
                out=bv_rep[:],
                in_=BV[:].rearrange("(h d) -> h d", h=HL)[None].to_broadcast(
                    (P, HL, HD)
                ),
            )
            for cc in range(2):
                nc.sync.dma_start(
                    out=wo_sb[:, cc, :],
                    in_=WOT[:].rearrange("(cc p) n -> p cc n", p=P)[:, cc, :],
                )
            nc.vector.memset(v_sb[:, :, :, HD : HD + 1], 1.0)

            # ---------------- Phase A: projections ----------------
            with (
                tc.tile_pool(name="wpool", bufs=1) as wpool,
                tc.tile_pool(name="panpool", bufs=2) as panpool,
                tc.tile_pool(name="pjpsum", bufs=4, space="PSUM") as pjpsum,
                tc.tile_pool(name="vpsum", bufs=4, space="PSUM") as vpsum,
            ):
                wq_sb = wpool.tile([P, KO, DL], BF16, tag="wq")
                wk_sb = wpool.tile([P, KO, DL], BF16, tag="wk")
                wv_sb = wpool.tile([P, KO, DL], BF16, tag="wv")
                for w_sb, W in ((wq_sb, WQT), (wk_sb, WKT), (wv_sb, WVT)):
                    nc.sync.dma_start(
                        out=w_sb[:], in_=W[:].rearrange("(ko p) m -> p ko m", p=P)
                    )

                for pan in range(NPAN):
                    qs = slice(pan * PANW, (pan + 1) * PANW)
                    qt_pan = panpool.tile([P, KO, PANW], BF16, tag="qt_pan")
                    kt_pan = panpool.tile([P, KO, PANW], BF16, tag="kt_pan")
                    for ko in range(KO):
                        nc.sync.dma_start(out=qt_pan[:, ko, :], in_=qt_r[:, ko, qs])
                        nc.sync.dma_start(out=kt_pan[:, ko, :], in_=kt_r[:, ko, qs])

                    # Q_^T and K_^T (feature-on-partition), bias fused in evict
                    for pan_in, w_sb, b_sb, dst in (
                        (qt_pan, wq_sb, bq_sb, qT),
                        (kt_pan, wk_sb, bk_sb, kT),
                    ):
                        for dc in range(2):
                            ps = pjpsum.tile([P, PANW], F32, tag="pj")
                            for ko in range(KO):
                                nc.tensor.matmul(
                                    ps[:],
                                    lhsT=(w_sb[:, ko, dc * P : (dc + 1) * P]),
                                    rhs=(pan_in[:, ko, :]),
                                    start=(ko == 0),
                                    stop=(ko == KO - 1),
                                )
                            nc.scalar.activation(
                                out=dst[:, dc, qs],
                                in_=ps[:],
                                func=AF.Identity,
                                bias=b_sb[:, dc : dc + 1],
                                scale=1.0,
                            )

                    # V natural layout (token-on-partition), bias via DVE add
                    for t4 in range(PANW // P):
                        tci = pan * (PANW // P) + t4
                        psv = vpsum.tile([P, DL], F32, tag="pv")
                        for ko in range(KO):
                            nc.tensor.matmul(
                                psv[:],
                                lhsT=(
                                    kt_pan[:, ko, t4 * P : (t4 + 1) * P]
                                ),
                                rhs=(wv_sb[:, ko, :]),
                                start=(ko == 0),
                                stop=(ko == KO - 1),
                            )
                        nc.vector.tensor_add(
                            out=v_sb[:, tci, :, 0:HD],
                            in0=psv[:].rearrange("p (h d) -> p h d", h=HL),
                            in1=bv_rep[:],
                        )

                # mask load last so it fills DMA gaps during phase A
                for kc in range(NKC):
                    nc.sync.dma_start(out=mT[:, kc, :], in_=mt_r[:, kc, :])

            # ---------------- Phase B: attention ----------------
            oT = otpool.tile([P, 2, N], BF16)
            with (
                tc.tile_pool(name="expool", bufs=2) as expool,
                tc.tile_pool(name="srpool", bufs=2) as srpool,
                tc.tile_pool(name="spsum", bufs=2, space="PSUM") as spsum,
                tc.tile_pool(name="opsum", bufs=2, space="PSUM") as opsum,
            ):
                for h in range(HL):
                    dc, po = h // 2, (h % 2) * HD
                    for qg in range(N // 1024):
                        ex = expool.tile([P, NKC, 1024], BF16, tag="ex")
                        for kc in range(NKC):
                            ps = spsum.tile([P, 1024], F32, tag="es")
                            for half in range(2):
                                q0 = qg * 1024 + half * 512
                                nc.tensor.matmul(
                                    ps[:, half * 512 : (half + 1) * 512],
                                    lhsT=kT[po : po + HD, dc, kc * P : (kc + 1) * P],
                                    rhs=qT[po : po + HD, dc, q0 : q0 + 512],
                                    start=True,
                                    stop=True,
                                )
                            nc.scalar.activation(
                                out=ex[:, kc, :], in_=ps[:], func=AF.Exp, scale=SCALE
                            )
                            nc.vector.tensor_mul(
                                out=ex[:, kc, :],
                                in0=ex[:, kc, :],
                                in1=mT[:, kc, qg * 1024 : (qg + 1) * 1024],
                            )
                        for qbh in range(2):
                            pso = opsum.tile([HD + 1, 512], F32, tag="pvo")
                            for kc in range(NKC):
                                nc.tensor.matmul(
                                    pso[:],
                                    lhsT=v_sb[:, kc, h, :],
                                    rhs=ex[:, kc, qbh * 512 : (qbh + 1) * 512],
                                    start=(kc == 0),
                                    stop=(kc == NKC - 1),
                                )
                            s_row = srpool.tile([1, 512], F32, tag="srow")
                            nc.scalar.copy(out=s_row[:], in_=pso[HD : HD + 1, :])
                            srp = opsum.tile([HD, 512], F32, tag="srp")
                            nc.tensor.matmul(
                                srp[:],
                                lhsT=ones_sb[:],
                                rhs=s_row[:],
                                start=True,
                                stop=True,
                            )
                            s_rep = srpool.tile([HD, 512], F32, tag="srep")
                            nc.vector.reciprocal(out=s_rep[:], in_=srp[:])
                            o_tmp = srpool.tile([HD, 512], BF16, tag="otmp")
                            nc.vector.tensor_mul(
                                out=o_tmp[:], in0=pso[0:HD, :], in1=s_rep[:]
                            )
                            q0 = qg * 1024 + qbh * 512
                            nc.sync.dma_start(
                                out=oT[po : po + HD, dc, q0 : q0 + 512], in_=o_tmp[:]
                            )

            # ---------------- Phase C: output projection ----------------
            with (
                tc.tile_pool(name="cout", bufs=3) as cout,
                tc.tile_pool(name="cpsum", bufs=4, space="PSUM") as cpsum,
            ):
                for qc in range(NQC):
                    pss = [
                        cpsum.tile([P, 512], F32, tag="co", name=f"co{i}")
                        for i in range(2)
                    ]
                    for cc in range(2):
                        for nh in range(2):
                            nc.tensor.matmul(
                                pss[nh][:],
                                lhsT=(oT[:, cc, qc * P : (qc + 1) * P]),
                                rhs=(wo_sb[:, cc, nh * 512 : (nh + 1) * 512]),
                                start=(cc == 0),
                                stop=(cc == 1),
                            )
                    o_sb = cout.tile([P, D], F32, tag="osb")
                    for nh in range(2):
                        nc.vector.tensor_copy(
                            out=o_sb[:, nh * 512 : (nh + 1) * 512], in_=pss[nh][:]
                        )
                    nc.sync.dma_start(out=OUT[qc * P : (qc + 1) * P, :], in_=o_sb[:])

    nc.finalize()
    return nc


_NC = None


def _get_nc():
    global _NC
    if _NC is None:
        _NC = build_nc()
    return _NC


def make_in_maps(Q, K, mask, Wq, bq, Wk, bk, Wv, bv, Wo, bo):
    Q = np.asarray(Q, np.float32)
    K = np.asarray(K, np.float32)
    mask = np.asarray(mask)
    Wq = np.asarray(Wq, np.float32)
    Wk = np.asarray(Wk, np.float32)
    Wv = np.asarray(Wv, np.float32)
    Wo = np.asarray(Wo, np.float32)
    qt = [np.ascontiguousarray(Q[b].T).astype(ml_dtypes.bfloat16) for b in range(B)]
    kt = [np.ascontiguousarray(K[b].T).astype(ml_dtypes.bfloat16) for b in range(B)]
    mt = [
        np.ascontiguousarray(mask[b].T).astype(ml_dtypes.bfloat16) for b in range(B)
    ]
    in_maps = []
    for c in range(8):
        b, hg = divmod(c, 4)
        cols = slice(hg * DL, (hg + 1) * DL)
        in_maps.append(
            {
                "qt": qt[b],
                "kt": kt[b],
                "mt": mt[b],
                "wqt": np.ascontiguousarray(Wq[cols, :].T).astype(ml_dtypes.bfloat16),
                "wkt": np.ascontiguousarray(Wk[cols, :].T).astype(ml_dtypes.bfloat16),
                "wvt": np.ascontiguousarray(Wv[cols, :].T).astype(ml_dtypes.bfloat16),
                "wot": np.ascontiguousarray(Wo[:, cols].T).astype(ml_dtypes.bfloat16),
                "bq": np.ascontiguousarray(np.asarray(bq, np.float32)[cols]),
                "bk": np.ascontiguousarray(np.asarray(bk, np.float32)[cols]),
                "bv": np.ascontiguousarray(np.asarray(bv, np.float32)[cols]),
            }
        )
    return in_maps


def assemble(results, bo):
    O = np.zeros((B, N, D), np.float32)
    for c in range(8):
        b = c // 4
        O[b] += results[c]["out"]
    O += np.asarray(bo, np.float32)[None, None, :]
    return O


def kernel(Q, K, mask, Wq, bq, Wk, bk, Wv, bv, Wo, bo):
    nc = _get_nc()
    in_maps = make_in_maps(Q, K, mask, Wq, bq, Wk, bk, Wv, bv, Wo, bo)
    res = run_bass_kernel_spmd(nc, in_maps, core_ids=list(range(8)))
    return assemble(res.results, bo)

